# revision 36
# baseline (speedup 1.0000x reference)
"""Performer cross-attention Trainium2 kernel (8 NeuronCores).

Self-contained: hardcodes B=4, L=4096, D=1024, NHEAD=16, D_HEAD=64.

Sharding: core c handles batch c//2, query-token half c%2 (2048 q-tokens).
Each core (v1) computes the full per-batch kv state from all 4096 k-tokens
(duplicated within the core pair), then its 2048 output rows.

Math (per batch b, head h):
  qh.T = Wq.T^T-contracted projections computed feature-major [D, T]
  kh,vh computed token-major [T, D]
  kf = exp(min(kh*m,0)) + relu(kh*m), qf likewise (elu(x)+1)
  kv_aug = kf_h^T @ [vh_h | 1]           -> [64, 65] (col 64 = k_sum)
  outT_aug = kv_aug^T-contracted vs qfT  -> [65, T] (row 64 = denom)
  attnT = outT[:64] * (1/(denom+eps)) broadcast via K=1 matmul
  out = attnT^T-contracted with Wo.T, token-major [T, D]
"""

import numpy as np
import ml_dtypes

import concourse.bass as bass
import concourse.bacc as bacc
import concourse.mybir as mybir
import concourse.tile as tile_mod
from concourse.tile import TileContext
from concourse.vector_clock import ScopedClock
from concourse.bass_utils import run_bass_kernel_spmd
from concourse.tile_rust import add_dep_helper

BF16 = mybir.dt.bfloat16
F32 = mybir.dt.float32
AF = mybir.ActivationFunctionType

B, L, D, NH, DH = 4, 4096, 1024, 16, 64
EPS = 1e-6
N_CORES = 8
TQ = L // 2          # q tokens per core (2048)
P = 128

_drain_patch_done = False


def _install_drain_patch():
    """walrus in this image caps sync-waits per instruction; split the Tile
    epilogue drain into one drain per wait."""
    global _drain_patch_done
    if _drain_patch_done:
        return
    _drain_patch_done = True

    def _patched(self, tick_clock, wait_clock):
        nc = self.nc
        drain_inst = nc.sync.drain()
        wait_clock.add_sem_waits(
            drain_inst.ins, ScopedClock({None: tick_clock.global_clock})
        )
        si = drain_inst.ins.sync_info
        if si is not None:
            waits = list(si.on_wait)
            if len(waits) > 1:
                upd = list(si.on_update)
                drain_inst.ins.sync_info = mybir.SyncInfo(
                    on_wait=waits[:1], on_update=upd
                )
                for w in waits[1:]:
                    extra = nc.sync.drain()
                    extra.ins.sync_info = mybir.SyncInfo(on_wait=[w], on_update=[])
        nc.all_engine_barrier()
        popped = nc._tile_sem_poison_stack.pop()
        assert popped is self._sem_poison
        nc.clear_and_free_semaphores(list(self.sems.allocated().values()))
        nc.all_engine_barrier()

    tile_mod.TileContext._drain_and_barrier = _patched


def build_program(d=D, nh=NH, tq=TQ, tk=TQ, has_bias=False, has_mask=True,
                  replica_groups=((0, 1), (2, 3), (4, 5), (6, 7))):
    """Build the single-core SPMD Bass program.

    d: model dim, nh: heads, tq: q tokens per core, tk: k tokens per core.
    """
    assert d % P == 0 and nh % 2 == 0
    IO = d // P              # contraction tiles (8)
    OBW = min(512, d)        # output block width
    OB = d // OBW            # output blocks (2)
    NP = nh // 2             # head pairs (8)
    NTQ = max(1, tq // 512)  # q-token blocks of 512
    TQB = min(tq, 512)       # q block width
    NST = tk // 256          # k/v super-tiles of 256 tokens
    dh = d // nh
    assert dh == 64

    nc = bacc.Bacc("TRN2", num_devices=N_CORES)

    # inputs pre-tiled on the host to the exact SBUF tile layouts so each
    # DMA descriptor covers one partition's full row (4-16KB contiguous)
    qH = nc.dram_tensor("qH", [NTQ, P, IO, TQB], BF16, kind="ExternalInput")
    kH = nc.dram_tensor("kH", [NST, P, IO, 256], BF16, kind="ExternalInput")
    vH = nc.dram_tensor("vH", [NST, P, IO, 256], BF16, kind="ExternalInput")
    wqT = nc.dram_tensor("wqT", [P, IO, d], BF16, kind="ExternalInput")
    wkT = nc.dram_tensor("wkT", [P, IO, d], BF16, kind="ExternalInput")
    wvT = nc.dram_tensor("wvT", [P, IO, d], BF16, kind="ExternalInput")
    woT = nc.dram_tensor("woT", [P, IO, d], BF16, kind="ExternalInput")
    maskf = nc.dram_tensor("maskf", [tk], F32, kind="ExternalInput")
    eh = nc.dram_tensor("eh", [nh, nh, 64], mybir.dt.float32r, kind="ExternalInput")
    out = nc.dram_tensor("out", [tq // P, P, d], F32, kind="ExternalOutput")

    mask_r = maskf.rearrange("(t p) -> p t", p=P)

    with TileContext(nc) as tc:
        with (
            tc.tile_pool(name="wpool", bufs=3) as wpool,
            tc.tile_pool(name="io_pool", bufs=2) as io_pool,
            tc.tile_pool(name="act_pool", bufs=2) as act_pool,
            tc.tile_pool(name="small", bufs=1) as small,
            tc.tile_pool(name="dram", bufs=1, space="DRAM") as dram,
            tc.tile_pool(name="psum", bufs=1, space="PSUM") as psum,
        ):
            # ---- resident constants ----
            w_q = wpool.tile([P, IO, d], BF16, tag="wmat", name="w_q")
            w_k = wpool.tile([P, IO, d], BF16, tag="wmat", name="w_k")
            w_v = wpool.tile([P, IO, d], BF16, tag="wmat", name="w_v")
            nc.sync.dma_start(out=w_k, in_=wkT[:])
            nc.sync.dma_start(out=w_v, in_=wvT[:])
            nc.sync.dma_start(out=w_q, in_=wqT[:])

            mask_sb = small.tile([P, tk // P], F32, name="mask_sb")
            if has_mask:
                nc.sync.dma_start(out=mask_sb, in_=mask_r)

            kv_sb = small.tile([P, NP, 130], BF16, name="kv_sb")
            ksum_mat = small.tile([P, IO, nh], BF16, name="ksum_mat")
            # fp32 accumulator for kv across super-tiles (psum groups must
            # close within a bank, so cross-tile accumulation lives in SBUF)
            kv_acc = small.tile([P, NP, 130], F32, name="kv_acc")
            nc.vector.memset(kv_acc, 0.0)
            n_banks = (NP + 2) // 3

            def feature_map(ps, dst, mcol, bcol):
                """dst(bf16) = exp(min(x,0)) + relu(x), x = ps*mcol (+bcol).

                Single PSUM reader (the copy): same-bank PSUM readers get
                serialized by the bank tracker, which throttles how fast the
                projection PSUM slots recycle and stalls the PE.
                """
                W = ps.shape[-1]
                tmp = act_pool.tile([P, W], F32, tag="fm_tmp", name="fm_tmp")
                tmpb = act_pool.tile([P, 2 * W], BF16, tag="fm_tmpb", name="fm_tmpb")
                t_raw, t_min, t_rel = tmp[:, :W], tmpb[:, :W], tmpb[:, W:]
                if mcol is not None:
                    nc.vector.tensor_scalar_mul(t_raw, ps, mcol)
                elif bcol is not None:
                    nc.vector.tensor_scalar_add(t_raw, ps, bcol)
                else:
                    nc.vector.tensor_copy(out=t_raw, in_=ps)
                min_inst = nc.vector.tensor_scalar_min(t_min, t_raw, 0.0)
                nc.scalar.activation(t_rel, t_raw, AF.Relu)
                nc.scalar.activation(t_min, t_min, AF.Exp)
                # gpsimd: both inputs in SBUF, keeps the busy DVE free
                nc.gpsimd.tensor_add(out=dst, in0=t_min, in1=t_rel)
                return min_inst

            kv_pend = {}

            def emit_kv(st_i):
                kf_i, vh_i = kv_pend.pop(st_i)
                for bi in range(n_banks):
                    pairs = min(3, NP - bi * 3)
                    kv_t = psum.tile(
                        [P, pairs * 130], F32, tag="misc", bufs=3, name=f"kv_t{bi}"
                    )
                    for pj in range(pairs):
                        hp = bi * 3 + pj
                        for sub in range(2):
                            nc.tensor.matmul(
                                kv_t[:, pj * 130:(pj + 1) * 130],
                                lhsT=kf_i[:, sub, hp * 128:(hp + 1) * 128],
                                rhs=vh_i[:, sub, hp, :],
                                start=(sub == 0),
                                stop=(sub == 1),
                            )
                    acc = kv_acc[:, bi * 3: bi * 3 + pairs, :]
                    nc.vector.tensor_add(
                        out=acc,
                        in0=acc,
                        in1=kv_t.rearrange("p (hp x) -> p hp x", x=130),
                    )

            # prefetch all q blocks up front (their DMAs otherwise queue
            # behind 16MB of k/v traffic and stall stage B)
            q_tiles = {}
            QPRE = min(3, NTQ)
            for tqb in range(QPRE):
                q_sb = io_pool.tile(
                    [P, IO, TQB], BF16, tag="q_sb", bufs=QPRE, name="q_sb"
                )
                nc.sync.dma_start(out=q_sb, in_=qH[tqb])
                q_tiles[tqb] = q_sb

            # =========== stage A: k/v side ===========
            for st in range(NST):
                k_sb = io_pool.tile([P, IO, 256], BF16, tag="k_sb", name="k_sb")
                v_sb = io_pool.tile([P, IO, 256], BF16, tag="v_sb", name="v_sb")
                nc.sync.dma_start(out=k_sb, in_=kH[st])
                nc.sync.dma_start(out=v_sb, in_=vH[st])
                kf_sb = act_pool.tile([P, 2, d], BF16, tag="kf", bufs=3, name="kf_sb")
                vh_sb = act_pool.tile([P, 2, NP, 130], BF16, tag="vh", bufs=3, name="vh_sb")
                nc.vector.memset(vh_sb[:, :, :, 64:65], 1.0)
                nc.vector.memset(vh_sb[:, :, :, 129:130], 1.0)
                for sub in range(2):
                    t = st * 2 + sub  # 128-token tile index
                    mcol = mask_sb[:, t:t + 1] if has_mask else None
                    for ob in range(OB):
                        # k projection: [T=128, O=OBW]
                        ps_k = psum.tile([P, OBW], F32, tag="proj", bufs=3, name="ps_k")
                        for io in range(IO):
                            nc.tensor.matmul(
                                ps_k,
                                lhsT=k_sb[:, io, sub * 128:(sub + 1) * 128],
                                rhs=w_k[:, io, ob * OBW:(ob + 1) * OBW],
                                start=(io == 0),
                                stop=(io == IO - 1),
                            )
                        feature_map(
                            ps_k, kf_sb[:, sub, ob * OBW:(ob + 1) * OBW], mcol, None
                        )

                        # v projection
                        ps_v = psum.tile([P, OBW], F32, tag="proj", bufs=3, name="ps_v")
                        for io in range(IO):
                            nc.tensor.matmul(
                                ps_v,
                                lhsT=v_sb[:, io, sub * 128:(sub + 1) * 128],
                                rhs=w_v[:, io, ob * OBW:(ob + 1) * OBW],
                                start=(io == 0),
                                stop=(io == IO - 1),
                            )
                        # scatter into vh_sb (even/odd head halves of each
                        # pair land at col 0 / 65) — one strided op, single
                        # PSUM read
                        npb = OBW // 128  # pairs per block
                        ps_v4 = ps_v.rearrange("p (hp two x) -> p hp two x", two=2, x=64)
                        po = ob * npb
                        dst = vh_sb[:, sub, po:po + npb, 0:130].rearrange(
                            "p h (two y) -> p h two y", y=65
                        )[:, :, :, 0:64]
                        if has_mask:
                            nc.vector.tensor_scalar_mul(dst, ps_v4, mcol)
                        else:
                            nc.vector.tensor_copy(out=dst, in_=ps_v4)

                # kv matmuls run one super-tile behind the projections so
                # the PE never waits on the feature-map chain tail
                kv_pend[st] = (kf_sb, vh_sb)
                if st > 0:
                    emit_kv(st - 1)
                if st == NST - 1:
                    emit_kv(st)

            # pair kv merge via AllGather (cheaper than AllReduce at this
            # size): trigger now; the consume side (gather-sum + ksum
            # extraction) is emitted AFTER the B stages so no engine queue
            # holds collective-dependent work ahead of independent work.
            cc_out = None
            if replica_groups is not None:
                kv_pre = small.tile([P, NP, 130], BF16, name="kv_pre")
                nc.vector.tensor_copy(out=kv_pre, in_=kv_acc)
                cc_in = dram.tile([P, NP, 130], BF16, name="cc_in")
                cc_out = dram.tile([2 * P, NP, 130], BF16, name="cc_out")
                nc.sync.dma_start(out=cc_in, in_=kv_pre)
                nc.gpsimd.collective_compute(
                    "AllGather",
                    mybir.AluOpType.bypass,
                    replica_groups=[list(g) for g in replica_groups],
                    ins=[cc_in.opt()],
                    outs=[cc_out.opt()],
                )

            def merge_kv():
                anchor = stage_lastdve.get(max(0, NTQ - 2))
                if cc_out is not None:
                    kv_g0 = small.tile([P, NP, 130], BF16, name="kv_g0")
                    kv_g1 = small.tile([P, NP, 130], BF16, name="kv_g1")
                    nc.sync.dma_start(out=kv_g0, in_=cc_out[0:P])
                    nc.sync.dma_start(out=kv_g1, in_=cc_out[P:2 * P])
                    mi = nc.vector.tensor_add(out=kv_sb, in0=kv_g0, in1=kv_g1)
                else:
                    mi = nc.vector.tensor_copy(out=kv_sb, in_=kv_acc)
                if anchor is not None:
                    # keep the collective-dependent merge out of the B-stage
                    # DVE stream (scheduler would otherwise hoist it and
                    # stall the feature-map chain on the collective)
                    add_dep_helper(mi.ins, anchor.ins, False,
                                   "merge after B-stage DVE work")

                # block-diagonal k_sum matrix [d, nh]: column h holds
                # k_sum_h in head h's row range, zeros elsewhere. Gives all
                # heads' denominators from IO accumulating matmuls into one
                # [nh, TQB] psum tile (one batched reciprocal).
                nc.vector.memset(ksum_mat, 0.0)
                for h in range(nh):
                    pb = (h % 2) * 64
                    hp = h // 2
                    col = (h % 2) * 65 + 64
                    nc.vector.tensor_copy(
                        out=ksum_mat[pb:pb + 64, hp, h:h + 1],
                        in_=kv_sb[pb:pb + 64, hp, col:col + 1],
                    )

            # output-projection weights (slot-cycles after stage A's w_k dies)
            w_o = wpool.tile([P, IO, d], BF16, tag="wmat", name="w_o")
            nc.sync.dma_start(out=w_o, in_=woT[:])

            # indicator rows for the per-head reciprocal broadcast matmul:
            # eh_all[:, h, :] is [nh, 64] with row h all-ones (host-provided).
            eh_all = small.tile([nh, nh, 64], mybir.dt.float32r, name="eh_all")
            nc.sync.dma_start(out=eh_all, in_=eh[:])

            # =========== stages B/C/D per q block ===========
            # Software-pipelined: emit stage B + denominator matmuls of
            # block tq before stages C'/D of block tq-1, so the PE engine
            # queue always has projection work while the (serial) DVE
            # reciprocal for the current block is in flight.
            stageB_out = {}
            stageD_out = {}
            stage_lastdve = {}

            def stage_B(tqb):
                if tqb in q_tiles:
                    q_sb = q_tiles.pop(tqb)
                else:
                    q_sb = io_pool.tile(
                        [P, IO, TQB], BF16, tag="q_sb", bufs=QPRE, name="q_sb"
                    )
                    nc.sync.dma_start(out=q_sb, in_=qH[tqb])

                qf_sb = act_pool.tile([P, IO, TQB], BF16, tag="qf", bufs=4, name="qf_sb")

                # B: q projection, feature-major [O=128, T=TQB]
                for o in range(IO):
                    ps_q = psum.tile([P, TQB], F32, tag="proj", bufs=3, name="ps_q")
                    for io in range(IO):
                        nc.tensor.matmul(
                            ps_q,
                            lhsT=w_q[:, io, o * 128:(o + 1) * 128],
                            rhs=q_sb[:, io, :],
                            start=(io == 0),
                            stop=(io == IO - 1),
                        )
                    last_fm = feature_map(ps_q, qf_sb[:, o, :], None, None)
                stageB_out[tqb] = qf_sb
                stage_lastdve[tqb] = last_fm

            def stage_den(tqb):
                qf_sb = stageB_out[tqb]
                ps_d = psum.tile([nh, TQB], F32, tag="den", bufs=2, name="ps_d")
                for j in range(IO):
                    nc.tensor.matmul(
                        ps_d,
                        lhsT=ksum_mat[:, j, :],
                        rhs=qf_sb[:, j, :],
                        start=(j == 0),
                        stop=(j == IO - 1),
                    )
                den_sb = act_pool.tile([nh, TQB], F32, tag="den", bufs=3, name="den_sb")
                rcp_sb = act_pool.tile(
                    [nh, TQB], mybir.dt.float32r, tag="rcp", bufs=3, name="rcp_sb"
                )
                nc.vector.tensor_scalar_add(den_sb, ps_d, EPS)
                with nc.allow_low_precision(reason="f32r recip for bcast mm"):
                    nc.vector.reciprocal(rcp_sb, den_sb)
                stageD_out[tqb] = rcp_sb

            def stage_CD(tqb):
                qf_sb = stageB_out.pop(tqb)
                rcp_sb = stageD_out.pop(tqb)
                at_sb = act_pool.tile([P, IO, TQB], BF16, tag="at", name="at_sb")

                # C': per-head numerator and division
                for h in range(nh):
                    pb = (h % 2) * 64
                    hp = h // 2
                    kv_lhsT = kv_sb[pb:pb + 64, hp, (h % 2) * 65:(h % 2) * 65 + 64]
                    qf_h = qf_sb[pb:pb + 64, hp, :]
                    ps_o = psum.tile([64, TQB], F32, tag="misc", bufs=3, name="ps_o")
                    nc.tensor.matmul(ps_o, lhsT=kv_lhsT, rhs=qf_h)

                    ps_b = psum.tile([64, TQB], F32, tag="misc", bufs=3, name="ps_b")
                    nc.tensor.matmul(ps_b, lhsT=eh_all[:, h, :], rhs=rcp_sb)
                    num_sb = act_pool.tile([64, TQB], BF16, tag="num", name="num_sb")
                    nc.any.tensor_copy(out=num_sb, in_=ps_o)
                    nc.vector.tensor_mul(
                        out=at_sb[pb:pb + 64, hp, :], in0=num_sb, in1=ps_b
                    )

                # D: output projection, token-major
                for tf in range(TQB // 128):
                    o_sb = act_pool.tile([P, d], F32, tag="o_sb", name="o_sb")
                    for ob in range(OB):
                        ps_f = psum.tile([P, OBW], F32, tag="proj", bufs=3, name="ps_f")
                        for cb in range(IO):
                            nc.tensor.matmul(
                                ps_f,
                                lhsT=at_sb[:, cb, tf * 128:(tf + 1) * 128],
                                rhs=w_o[:, cb, ob * OBW:(ob + 1) * OBW],
                                start=(cb == 0),
                                stop=(cb == IO - 1),
                            )
                        nc.any.tensor_copy(
                            out=o_sb[:, ob * OBW:(ob + 1) * OBW], in_=ps_f
                        )
                    nc.sync.dma_start(
                        out=out[tqb * (TQB // 128) + tf], in_=o_sb
                    )

            # Pipelined schedule: all B stages first (they are the only
            # collective-independent PE work), then the kv merge, then
            # den/CD with a one-stage skew covering the reciprocal.
            for tqb in range(NTQ):
                stage_B(tqb)
            merge_kv()
            stage_den(0)
            for tqb in range(1, NTQ):
                stage_den(tqb)
                stage_CD(tqb - 1)
            stage_CD(NTQ - 1)

    nc.compile()
    return nc


def _tile_tok(x2d, block):
    """[T, d] -> [T//block, 128, d//128, block] (SBUF tile layout)."""
    T, d = x2d.shape
    nb, io = T // block, d // 128
    return np.ascontiguousarray(
        x2d.reshape(nb, block, io, 128).transpose(0, 3, 2, 1)
    )


def _tile_w(w):
    """[out, in] -> [128, in//128, out] (w.T tiled for SBUF)."""
    o, i = w.shape
    return np.ascontiguousarray(w.T.reshape(i // 128, 128, o).transpose(1, 0, 2))


def _to_np(x):
    a = np.asarray(x)
    return a


_cached = {}


def _get_program():
    if "nc" not in _cached:
        _cached["nc"] = build_program()
    return _cached["nc"]


def kernel(q, k, v, key_padding_mask, Wq, bq, Wk, bk, Wv, bv, Wo, bo, **kw):
    q = _to_np(q).astype(np.float32, copy=False)
    k = _to_np(k).astype(np.float32, copy=False)
    v = _to_np(v).astype(np.float32, copy=False)
    key_padding_mask = _to_np(key_padding_mask)
    Wq, Wk, Wv, Wo = (_to_np(w).astype(np.float32, copy=False) for w in (Wq, Wk, Wv, Wo))
    bq, bk, bv, bo = (_to_np(b).astype(np.float32, copy=False) for b in (bq, bk, bv, bo))
    assert not (np.any(bq) or np.any(bk) or np.any(bv) or np.any(bo)), (
        "nonzero biases not supported by this kernel build"
    )

    bf = ml_dtypes.bfloat16
    eh_mat = np.zeros((NH, NH, 64), dtype=np.float32)
    for h in range(NH):
        eh_mat[h, h, :] = 1.0

    wqT = _tile_w(Wq).astype(bf)
    wkT = _tile_w(Wk).astype(bf)
    wvT = _tile_w(Wv).astype(bf)
    woT = _tile_w(Wo).astype(bf)

    in_maps = []
    for c in range(N_CORES):
        b, half = divmod(c, 2)
        sl = slice(half * TQ, (half + 1) * TQ)
        in_maps.append(
            {
                "qH": _tile_tok(q[b, sl, :], 512).astype(bf),
                "kH": _tile_tok(k[b, sl, :], 256).astype(bf),
                "vH": _tile_tok(v[b, sl, :], 256).astype(bf),
                "wqT": wqT,
                "wkT": wkT,
                "wvT": wvT,
                "woT": woT,
                "maskf": (~key_padding_mask[b, sl]).astype(np.float32),
                "eh": eh_mat,
            }
        )

    import os

    nc = _get_program()
    trace = bool(os.environ.get("KERNEL_TRACE"))
    res = run_bass_kernel_spmd(
        nc, in_maps, core_ids=list(range(N_CORES)), trace=trace
    )
    _cached["last_results"] = res

    full = np.empty((B, L, D), dtype=np.float32)
    for c in range(N_CORES):
        b, half = divmod(c, 2)
        full[b, half * TQ:(half + 1) * TQ, :] = res.results[c]["out"].reshape(TQ, D)
    return full


# revision 37
# speedup vs baseline: 1.1359x; 1.1359x over previous
"""Performer cross-attention Trainium2 kernel (8 NeuronCores).

Self-contained: hardcodes B=4, L=4096, D=1024, NHEAD=16, D_HEAD=64.

Sharding: core c handles batch c//2, query-token half c%2 (2048 q-tokens).
Each core (v1) computes the full per-batch kv state from all 4096 k-tokens
(duplicated within the core pair), then its 2048 output rows.

Math (per batch b, head h):
  qh.T = Wq.T^T-contracted projections computed feature-major [D, T]
  kh,vh computed token-major [T, D]
  kf = exp(min(kh*m,0)) + relu(kh*m), qf likewise (elu(x)+1)
  kv_aug = kf_h^T @ [vh_h | 1]           -> [64, 65] (col 64 = k_sum)
  outT_aug = kv_aug^T-contracted vs qfT  -> [65, T] (row 64 = denom)
  attnT = outT[:64] * (1/(denom+eps)) broadcast via K=1 matmul
  out = attnT^T-contracted with Wo.T, token-major [T, D]
"""

import numpy as np
import ml_dtypes

import concourse.bass as bass
import concourse.bacc as bacc
import concourse.mybir as mybir
import concourse.tile as tile_mod
from concourse.tile import TileContext
from concourse.vector_clock import ScopedClock
from concourse.bass_utils import run_bass_kernel_spmd
from concourse.tile_rust import add_dep_helper

BF16 = mybir.dt.bfloat16
F32 = mybir.dt.float32
AF = mybir.ActivationFunctionType

B, L, D, NH, DH = 4, 4096, 1024, 16, 64
EPS = 1e-6
N_CORES = 8
TQ = L // 2          # q tokens per core (2048)
P = 128

_drain_patch_done = False


def _install_drain_patch():
    """walrus in this image caps sync-waits per instruction; split the Tile
    epilogue drain into one drain per wait."""
    global _drain_patch_done
    if _drain_patch_done:
        return
    _drain_patch_done = True

    def _patched(self, tick_clock, wait_clock):
        nc = self.nc
        drain_inst = nc.sync.drain()
        wait_clock.add_sem_waits(
            drain_inst.ins, ScopedClock({None: tick_clock.global_clock})
        )
        si = drain_inst.ins.sync_info
        if si is not None:
            waits = list(si.on_wait)
            if len(waits) > 1:
                upd = list(si.on_update)
                drain_inst.ins.sync_info = mybir.SyncInfo(
                    on_wait=waits[:1], on_update=upd
                )
                for w in waits[1:]:
                    extra = nc.sync.drain()
                    extra.ins.sync_info = mybir.SyncInfo(on_wait=[w], on_update=[])
        nc.all_engine_barrier()
        popped = nc._tile_sem_poison_stack.pop()
        assert popped is self._sem_poison
        nc.clear_and_free_semaphores(list(self.sems.allocated().values()))
        nc.all_engine_barrier()

    tile_mod.TileContext._drain_and_barrier = _patched


def build_program(d=D, nh=NH, tq=TQ, tk=TQ, has_bias=False, has_mask=True,
                  replica_groups=((0, 1), (2, 3), (4, 5), (6, 7))):
    """Build the single-core SPMD Bass program.

    d: model dim, nh: heads, tq: q tokens per core, tk: k tokens per core.
    """
    assert d % P == 0 and nh % 2 == 0
    IO = d // P              # contraction tiles (8)
    OBW = min(512, d)        # output block width
    OB = d // OBW            # output blocks (2)
    NP = nh // 2             # head pairs (8)
    NTQ = max(1, tq // 512)  # q-token blocks of 512
    TQB = min(tq, 512)       # q block width
    NST = tk // 256          # k/v super-tiles of 256 tokens
    dh = d // nh
    assert dh == 64

    nc = bacc.Bacc("TRN2", num_devices=N_CORES)

    # inputs pre-tiled on the host to the exact SBUF tile layouts so each
    # DMA descriptor covers one partition's full row (4-16KB contiguous)
    qH = nc.dram_tensor("qH", [NTQ, P, IO, TQB], BF16, kind="ExternalInput")
    kH = nc.dram_tensor("kH", [NST, P, IO, 256], BF16, kind="ExternalInput")
    vH = nc.dram_tensor("vH", [NST, P, IO, 256], BF16, kind="ExternalInput")
    wqT = nc.dram_tensor("wqT", [P, IO, d], BF16, kind="ExternalInput")
    wkT = nc.dram_tensor("wkT", [P, IO, d], BF16, kind="ExternalInput")
    wvT = nc.dram_tensor("wvT", [P, IO, d], BF16, kind="ExternalInput")
    woT = nc.dram_tensor("woT", [P, IO, d], BF16, kind="ExternalInput")
    maskf = nc.dram_tensor("maskf", [tk], F32, kind="ExternalInput")
    eh = nc.dram_tensor("eh", [nh, nh, 64], mybir.dt.float32r, kind="ExternalInput")
    out = nc.dram_tensor("out", [tq // P, P, d], F32, kind="ExternalOutput")

    mask_r = maskf.rearrange("(t p) -> p t", p=P)

    with TileContext(nc) as tc:
        with (
            tc.tile_pool(name="wpool", bufs=3) as wpool,
            tc.tile_pool(name="io_pool", bufs=2) as io_pool,
            tc.tile_pool(name="act_pool", bufs=2) as act_pool,
            tc.tile_pool(name="small", bufs=1) as small,
            tc.tile_pool(name="dram", bufs=1, space="DRAM") as dram,
            tc.tile_pool(name="psum", bufs=1, space="PSUM") as psum,
        ):
            # ---- resident constants ----
            w_q = wpool.tile([P, IO, d], BF16, tag="wmat", name="w_q")
            w_k = wpool.tile([P, IO, d], BF16, tag="wmat", name="w_k")
            w_v = wpool.tile([P, IO, d], BF16, tag="wmat", name="w_v")
            nc.sync.dma_start(out=w_k, in_=wkT[:])
            nc.sync.dma_start(out=w_v, in_=wvT[:])
            nc.sync.dma_start(out=w_q, in_=wqT[:])

            mask_sb = small.tile([P, tk // P], F32, name="mask_sb")
            if has_mask:
                nc.sync.dma_start(out=mask_sb, in_=mask_r)

            kv_sb = small.tile([P, NP, 130], BF16, name="kv_sb")
            ksum_mat = small.tile([P, IO, nh], BF16, name="ksum_mat")
            # fp32 accumulator for kv across super-tiles (psum groups must
            # close within a bank, so cross-tile accumulation lives in SBUF)
            kv_acc = small.tile([P, NP, 130], F32, name="kv_acc")
            nc.vector.memset(kv_acc, 0.0)
            n_banks = (NP + 2) // 3

            def feature_map(ps, dst, mcol, bcol):
                """dst(bf16) = exp(min(x,0)) + relu(x), x = ps*mcol (+bcol).

                Single PSUM reader (the copy): same-bank PSUM readers get
                serialized by the bank tracker, which throttles how fast the
                projection PSUM slots recycle and stalls the PE.
                """
                W = ps.shape[-1]
                tmp = act_pool.tile([P, W], F32, tag="fm_tmp", name="fm_tmp")
                tmpb = act_pool.tile([P, 2 * W], BF16, tag="fm_tmpb", name="fm_tmpb")
                t_raw, t_min, t_rel = tmp[:, :W], tmpb[:, :W], tmpb[:, W:]
                if mcol is not None:
                    nc.vector.tensor_scalar_mul(t_raw, ps, mcol)
                elif bcol is not None:
                    nc.vector.tensor_scalar_add(t_raw, ps, bcol)
                else:
                    nc.vector.tensor_copy(out=t_raw, in_=ps)
                min_inst = nc.vector.tensor_scalar_min(t_min, t_raw, 0.0)
                nc.scalar.activation(t_rel, t_raw, AF.Relu)
                nc.scalar.activation(t_min, t_min, AF.Exp)
                # gpsimd: both inputs in SBUF, keeps the busy DVE free
                nc.gpsimd.tensor_add(out=dst, in0=t_min, in1=t_rel)
                return min_inst

            kv_pend = {}

            def emit_kv(st_i):
                kf_i, vh_i = kv_pend.pop(st_i)
                for bi in range(n_banks):
                    pairs = min(3, NP - bi * 3)
                    kv_t = psum.tile(
                        [P, pairs * 130], F32, tag="misc", bufs=3, name=f"kv_t{bi}"
                    )
                    for pj in range(pairs):
                        hp = bi * 3 + pj
                        for sub in range(2):
                            nc.tensor.matmul(
                                kv_t[:, pj * 130:(pj + 1) * 130],
                                lhsT=kf_i[:, sub, hp * 128:(hp + 1) * 128],
                                rhs=vh_i[:, sub, hp, :],
                                start=(sub == 0),
                                stop=(sub == 1),
                            )
                    acc = kv_acc[:, bi * 3: bi * 3 + pairs, :]
                    nc.vector.tensor_add(
                        out=acc,
                        in0=acc,
                        in1=kv_t.rearrange("p (hp x) -> p hp x", x=130),
                    )

            # prefetch all q blocks up front (their DMAs otherwise queue
            # behind 16MB of k/v traffic and stall stage B)
            q_tiles = {}
            QPRE = min(3, NTQ)
            for tqb in range(QPRE):
                q_sb = io_pool.tile(
                    [P, IO, TQB], BF16, tag="q_sb", bufs=QPRE, name="q_sb"
                )
                nc.sync.dma_start(out=q_sb, in_=qH[tqb])
                q_tiles[tqb] = q_sb

            # =========== stage A: k/v side ===========
            for st in range(NST):
                k_sb = io_pool.tile([P, IO, 256], BF16, tag="k_sb", name="k_sb")
                v_sb = io_pool.tile([P, IO, 256], BF16, tag="v_sb", name="v_sb")
                nc.sync.dma_start(out=k_sb, in_=kH[st])
                nc.sync.dma_start(out=v_sb, in_=vH[st])
                kf_sb = act_pool.tile([P, 2, d], BF16, tag="kf", bufs=3, name="kf_sb")
                vh_sb = act_pool.tile([P, 2, NP, 130], BF16, tag="vh", bufs=3, name="vh_sb")
                nc.vector.memset(vh_sb[:, :, :, 64:65], 1.0)
                nc.vector.memset(vh_sb[:, :, :, 129:130], 1.0)
                for sub in range(2):
                    t = st * 2 + sub  # 128-token tile index
                    mcol = mask_sb[:, t:t + 1] if has_mask else None
                    # all OB output blocks accumulate together: the lhsT
                    # (activation tile) is reused by OB consecutive matmuls,
                    # halving LDWEIGHTS pressure
                    ps_ks = [
                        psum.tile([P, OBW], F32, tag="proj", bufs=4, name="ps_k")
                        for _ in range(OB)
                    ]
                    for io in range(IO):
                        for ob in range(OB):
                            nc.tensor.matmul(
                                ps_ks[ob],
                                lhsT=k_sb[:, io, sub * 128:(sub + 1) * 128],
                                rhs=w_k[:, io, ob * OBW:(ob + 1) * OBW],
                                start=(io == 0),
                                stop=(io == IO - 1),
                            )
                    for ob in range(OB):
                        feature_map(
                            ps_ks[ob], kf_sb[:, sub, ob * OBW:(ob + 1) * OBW],
                            mcol, None,
                        )

                    ps_vs = [
                        psum.tile([P, OBW], F32, tag="proj", bufs=4, name="ps_v")
                        for _ in range(OB)
                    ]
                    for io in range(IO):
                        for ob in range(OB):
                            nc.tensor.matmul(
                                ps_vs[ob],
                                lhsT=v_sb[:, io, sub * 128:(sub + 1) * 128],
                                rhs=w_v[:, io, ob * OBW:(ob + 1) * OBW],
                                start=(io == 0),
                                stop=(io == IO - 1),
                            )
                    for ob in range(OB):
                        ps_v = ps_vs[ob]
                        # scatter into vh_sb (even/odd head halves of each
                        # pair land at col 0 / 65) — one strided op, single
                        # PSUM read
                        npb = OBW // 128  # pairs per block
                        ps_v4 = ps_v.rearrange("p (hp two x) -> p hp two x", two=2, x=64)
                        po = ob * npb
                        dst = vh_sb[:, sub, po:po + npb, 0:130].rearrange(
                            "p h (two y) -> p h two y", y=65
                        )[:, :, :, 0:64]
                        if has_mask:
                            nc.vector.tensor_scalar_mul(dst, ps_v4, mcol)
                        else:
                            nc.vector.tensor_copy(out=dst, in_=ps_v4)

                # kv matmuls run one super-tile behind the projections so
                # the PE never waits on the feature-map chain tail
                kv_pend[st] = (kf_sb, vh_sb)
                if st > 0:
                    emit_kv(st - 1)
                if st == NST - 1:
                    emit_kv(st)

            # pair kv merge via AllGather (cheaper than AllReduce at this
            # size): trigger now; the consume side (gather-sum + ksum
            # extraction) is emitted AFTER the B stages so no engine queue
            # holds collective-dependent work ahead of independent work.
            cc_out = None
            if replica_groups is not None:
                kv_pre = small.tile([P, NP, 130], BF16, name="kv_pre")
                nc.vector.tensor_copy(out=kv_pre, in_=kv_acc)
                cc_in = dram.tile([P, NP, 130], BF16, name="cc_in")
                cc_out = dram.tile([2 * P, NP, 130], BF16, name="cc_out")
                nc.sync.dma_start(out=cc_in, in_=kv_pre)
                nc.gpsimd.collective_compute(
                    "AllGather",
                    mybir.AluOpType.bypass,
                    replica_groups=[list(g) for g in replica_groups],
                    ins=[cc_in.opt()],
                    outs=[cc_out.opt()],
                )

            def merge_kv():
                anchor = stage_lastdve.get(max(0, NTQ - 2))
                if cc_out is not None:
                    kv_g0 = small.tile([P, NP, 130], BF16, name="kv_g0")
                    kv_g1 = small.tile([P, NP, 130], BF16, name="kv_g1")
                    nc.sync.dma_start(out=kv_g0, in_=cc_out[0:P])
                    nc.sync.dma_start(out=kv_g1, in_=cc_out[P:2 * P])
                    mi = nc.vector.tensor_add(out=kv_sb, in0=kv_g0, in1=kv_g1)
                else:
                    mi = nc.vector.tensor_copy(out=kv_sb, in_=kv_acc)
                if anchor is not None:
                    # keep the collective-dependent merge out of the B-stage
                    # DVE stream (scheduler would otherwise hoist it and
                    # stall the feature-map chain on the collective)
                    add_dep_helper(mi.ins, anchor.ins, False,
                                   "merge after B-stage DVE work")

                # block-diagonal k_sum matrix [d, nh]: column h holds
                # k_sum_h in head h's row range, zeros elsewhere. Gives all
                # heads' denominators from IO accumulating matmuls into one
                # [nh, TQB] psum tile (one batched reciprocal).
                nc.vector.memset(ksum_mat, 0.0)
                for h in range(nh):
                    pb = (h % 2) * 64
                    hp = h // 2
                    col = (h % 2) * 65 + 64
                    nc.vector.tensor_copy(
                        out=ksum_mat[pb:pb + 64, hp, h:h + 1],
                        in_=kv_sb[pb:pb + 64, hp, col:col + 1],
                    )

            # output-projection weights (slot-cycles after stage A's w_k dies)
            w_o = wpool.tile([P, IO, d], BF16, tag="wmat", name="w_o")
            nc.sync.dma_start(out=w_o, in_=woT[:])

            # indicator rows for the per-head reciprocal broadcast matmul:
            # eh_all[:, h, :] is [nh, 64] with row h all-ones (host-provided).
            eh_all = small.tile([nh, nh, 64], mybir.dt.float32r, name="eh_all")
            nc.sync.dma_start(out=eh_all, in_=eh[:])

            # =========== stages B/C/D per q block ===========
            # Software-pipelined: emit stage B + denominator matmuls of
            # block tq before stages C'/D of block tq-1, so the PE engine
            # queue always has projection work while the (serial) DVE
            # reciprocal for the current block is in flight.
            stageB_out = {}
            stageD_out = {}
            stage_lastdve = {}

            def stage_B(tqb):
                if tqb in q_tiles:
                    q_sb = q_tiles.pop(tqb)
                else:
                    q_sb = io_pool.tile(
                        [P, IO, TQB], BF16, tag="q_sb", bufs=QPRE, name="q_sb"
                    )
                    nc.sync.dma_start(out=q_sb, in_=qH[tqb])

                qf_sb = act_pool.tile([P, IO, TQB], BF16, tag="qf", bufs=4, name="qf_sb")

                # B: q projection, feature-major [O=128, T=TQB]
                for o in range(IO):
                    ps_q = psum.tile([P, TQB], F32, tag="proj", bufs=4, name="ps_q")
                    for io in range(IO):
                        nc.tensor.matmul(
                            ps_q,
                            lhsT=w_q[:, io, o * 128:(o + 1) * 128],
                            rhs=q_sb[:, io, :],
                            start=(io == 0),
                            stop=(io == IO - 1),
                        )
                    last_fm = feature_map(ps_q, qf_sb[:, o, :], None, None)
                stageB_out[tqb] = qf_sb
                stage_lastdve[tqb] = last_fm

            def stage_den(tqb):
                qf_sb = stageB_out[tqb]
                ps_d = psum.tile([nh, TQB], F32, tag="den", bufs=1, name="ps_d")
                for j in range(IO):
                    nc.tensor.matmul(
                        ps_d,
                        lhsT=ksum_mat[:, j, :],
                        rhs=qf_sb[:, j, :],
                        start=(j == 0),
                        stop=(j == IO - 1),
                    )
                den_sb = act_pool.tile([nh, TQB], F32, tag="den", bufs=3, name="den_sb")
                rcp_sb = act_pool.tile(
                    [nh, TQB], mybir.dt.float32r, tag="rcp", bufs=3, name="rcp_sb"
                )
                nc.vector.tensor_scalar_add(den_sb, ps_d, EPS)
                with nc.allow_low_precision(reason="f32r recip for bcast mm"):
                    nc.vector.reciprocal(rcp_sb, den_sb)
                stageD_out[tqb] = rcp_sb

            def stage_CD(tqb):
                qf_sb = stageB_out.pop(tqb)
                rcp_sb = stageD_out.pop(tqb)
                at_sb = act_pool.tile([P, IO, TQB], BF16, tag="at", name="at_sb")

                # C': per-head numerator and division
                for h in range(nh):
                    pb = (h % 2) * 64
                    hp = h // 2
                    kv_lhsT = kv_sb[pb:pb + 64, hp, (h % 2) * 65:(h % 2) * 65 + 64]
                    qf_h = qf_sb[pb:pb + 64, hp, :]
                    ps_o = psum.tile([64, TQB], F32, tag="misc", bufs=3, name="ps_o")
                    nc.tensor.matmul(ps_o, lhsT=kv_lhsT, rhs=qf_h)

                    ps_b = psum.tile([64, TQB], F32, tag="misc", bufs=3, name="ps_b")
                    nc.tensor.matmul(ps_b, lhsT=eh_all[:, h, :], rhs=rcp_sb)
                    num_sb = act_pool.tile([64, TQB], BF16, tag="num", name="num_sb")
                    nc.any.tensor_copy(out=num_sb, in_=ps_o)
                    nc.vector.tensor_mul(
                        out=at_sb[pb:pb + 64, hp, :], in0=num_sb, in1=ps_b
                    )

                # D: output projection, token-major
                for tf in range(TQB // 128):
                    o_sb = act_pool.tile([P, d], F32, tag="o_sb", name="o_sb")
                    ps_fs = [
                        psum.tile([P, OBW], F32, tag="proj", bufs=4, name="ps_f")
                        for _ in range(OB)
                    ]
                    for cb in range(IO):
                        for ob in range(OB):
                            nc.tensor.matmul(
                                ps_fs[ob],
                                lhsT=at_sb[:, cb, tf * 128:(tf + 1) * 128],
                                rhs=w_o[:, cb, ob * OBW:(ob + 1) * OBW],
                                start=(cb == 0),
                                stop=(cb == IO - 1),
                            )
                    for ob in range(OB):
                        nc.any.tensor_copy(
                            out=o_sb[:, ob * OBW:(ob + 1) * OBW], in_=ps_fs[ob]
                        )
                    nc.sync.dma_start(
                        out=out[tqb * (TQB // 128) + tf], in_=o_sb
                    )

            # Pipelined schedule: all B stages first (they are the only
            # collective-independent PE work), then the kv merge, then
            # den/CD with a one-stage skew covering the reciprocal.
            for tqb in range(NTQ):
                stage_B(tqb)
            merge_kv()
            stage_den(0)
            for tqb in range(1, NTQ):
                stage_den(tqb)
                stage_CD(tqb - 1)
            stage_CD(NTQ - 1)

    nc.compile()
    return nc


def _tile_tok(x2d, block):
    """[T, d] -> [T//block, 128, d//128, block] (SBUF tile layout)."""
    T, d = x2d.shape
    nb, io = T // block, d // 128
    return np.ascontiguousarray(
        x2d.reshape(nb, block, io, 128).transpose(0, 3, 2, 1)
    )


def _tile_w(w):
    """[out, in] -> [128, in//128, out] (w.T tiled for SBUF)."""
    o, i = w.shape
    return np.ascontiguousarray(w.T.reshape(i // 128, 128, o).transpose(1, 0, 2))


def _to_np(x):
    a = np.asarray(x)
    return a


_cached = {}


def _get_program():
    if "nc" not in _cached:
        _cached["nc"] = build_program()
    return _cached["nc"]


def kernel(q, k, v, key_padding_mask, Wq, bq, Wk, bk, Wv, bv, Wo, bo, **kw):
    q = _to_np(q).astype(np.float32, copy=False)
    k = _to_np(k).astype(np.float32, copy=False)
    v = _to_np(v).astype(np.float32, copy=False)
    key_padding_mask = _to_np(key_padding_mask)
    Wq, Wk, Wv, Wo = (_to_np(w).astype(np.float32, copy=False) for w in (Wq, Wk, Wv, Wo))
    bq, bk, bv, bo = (_to_np(b).astype(np.float32, copy=False) for b in (bq, bk, bv, bo))
    assert not (np.any(bq) or np.any(bk) or np.any(bv) or np.any(bo)), (
        "nonzero biases not supported by this kernel build"
    )

    bf = ml_dtypes.bfloat16
    eh_mat = np.zeros((NH, NH, 64), dtype=np.float32)
    for h in range(NH):
        eh_mat[h, h, :] = 1.0

    wqT = _tile_w(Wq).astype(bf)
    wkT = _tile_w(Wk).astype(bf)
    wvT = _tile_w(Wv).astype(bf)
    woT = _tile_w(Wo).astype(bf)

    in_maps = []
    for c in range(N_CORES):
        b, half = divmod(c, 2)
        sl = slice(half * TQ, (half + 1) * TQ)
        in_maps.append(
            {
                "qH": _tile_tok(q[b, sl, :], 512).astype(bf),
                "kH": _tile_tok(k[b, sl, :], 256).astype(bf),
                "vH": _tile_tok(v[b, sl, :], 256).astype(bf),
                "wqT": wqT,
                "wkT": wkT,
                "wvT": wvT,
                "woT": woT,
                "maskf": (~key_padding_mask[b, sl]).astype(np.float32),
                "eh": eh_mat,
            }
        )

    import os

    nc = _get_program()
    trace = bool(os.environ.get("KERNEL_TRACE"))
    res = run_bass_kernel_spmd(
        nc, in_maps, core_ids=list(range(N_CORES)), trace=trace
    )
    _cached["last_results"] = res

    full = np.empty((B, L, D), dtype=np.float32)
    for c in range(N_CORES):
        b, half = divmod(c, 2)
        full[b, half * TQ:(half + 1) * TQ, :] = res.results[c]["out"].reshape(TQ, D)
    return full
